# revision 23
# baseline (speedup 1.0000x reference)
"""Single-head causal attention on 8 Trainium2 NeuronCores.

Problem: x:[8,2048,1024], Wq/Wk/Wv:[64,1024], bq/bk/bv:[64]
  q,k,v = x@W*.T + b*;  out = softmax(causal(q@k.T)/sqrt(64)) @ v

Sharding: batch dim (8) across the 8 cores - fully data-parallel, no
collectives. Each core computes one batch's attention head.

v2 design (all matmuls bf16, 1 cycle/row on the PE at any free size):
  - x and the packed projection weights are converted to bf16 on the host:
    halves HBM traffic and frees the fp32r >=256-free-dim constraint.
  - k/v projection stays h-major ([Wk|Wv] packed stationary, x moving:
    full 128-wide output). q projection runs s-major: x tiles double as
    stationary [128e,128s] blocks, Wq.T [128e,64] is the moving operand -
    64 rows per (e,s) pair instead of 512 per (e,chunk), 2x fewer PE rows
    for q. q then transposes back to h-major on the PE (bf16 identity).
  - causal dead-column skipping: for diagonal k-tiles only columns
    [128i, 512) are live; scores/exp/AV all operate on the live slice
    (saves ~2.5us PE + ~2.5us ACT). Only the [128,128] triangle block
    needs the 0/1 mask multiply (DVE, bf16).
  - ACT runs exp almost exclusively (the bottleneck engine in v1 at 82%);
    q/k epilogue copies live on DVE, v on ACT, outputs DMA straight from
    PSUM (no SBUF staging).
  - softmax without max-subtraction (scores/8 ~ N(0,1); exp safe in f32),
    denominator = ones-row appended to V, division on the host.
  - emission interleaves proj(c+1) with attn(c); the last chunk processes
    its (cheap, live-sliced) diagonal tiles first so the drain is fed by
    exps computed well in advance.
"""

import numpy as np
import ml_dtypes

import concourse.bacc as bacc
import concourse.mybir as mybir
import concourse.tile as tile
from concourse import bass2jax

B, S, E, H = 8, 2048, 1024, 64
NCORES = 8
PB = 128  # partition block / k-tile size
QB = 512  # q-chunk (matmul moving free dim)
ET = E // PB  # e-tiles per contraction
QC = S // QB  # q-chunks
KT = S // PB  # k-tiles
DIAG = QB // PB  # diagonal k-tiles per q-chunk

# packed bf16 constants layout: columns of the [128, NCONST] "consts" input
C_ID = 0  # [*, 0:128]    identity 128x128
C_TRI = 128  # [*, 128:256]  causal triangle: (p, j) = 1 iff j >= p
C_ONES = 256  # [*, 256:257]  ones
NCONST = 257
WQP = 2 * PB  # wqkv padded row length (bf16 512B descriptors, full DMA rate)
# f32 bias constants: columns of the [128, NBIAS] "biases" input
CB_K = 0  # k bias (rows 0:64)
CB_V = 1  # v bias (rows 64:128)
CB_Q = 2  # q bias / 8 (rows 0:64)
NBIAS = 3

F32 = mybir.dt.float32
BF16 = mybir.dt.bfloat16
AF = mybir.ActivationFunctionType
MUL = mybir.AluOpType.mult
ADD = mybir.AluOpType.add

_CACHE: dict = {}

# schedule/buffering knobs (sweepable)
CFG = {
    "lookahead": 7,
    "xbufs": 16,
    "wtbufs": 10,
    "psbufs": 3,
    "dma2": "gpsimd",  # second x-stream queue
    "cs_q": "gpsimd",
    "diag_first": True,
    "warmups": 58,  # PE p-state warmup matmuls (64-free) during startup DMA
}


def _interleave(*gens):
    """Drive generators round-robin; the first (proj) gets two steps per turn."""
    alive = list(gens)
    steps = {id(g): (2 if i == 0 and len(gens) > 1 else 1) for i, g in enumerate(gens)}
    while alive:
        for g in list(alive):
            for _ in range(steps[id(g)]):
                try:
                    next(g)
                except StopIteration:
                    alive.remove(g)
                    break


def _build_nc():
    nc = bacc.Bacc("TRN2", target_bir_lowering=False, debug=False)
    xT = nc.dram_tensor("xT", [E, S], BF16, kind="ExternalInput").ap()
    # cols 0:64 Wk.T, 64:128 Wv.T, 128:192 Wq.T, 192:256 zero padding
    wqkv = nc.dram_tensor("wqkv", [E, WQP], BF16, kind="ExternalInput").ap()
    consts = nc.dram_tensor("consts", [PB, NCONST], BF16, kind="ExternalInput").ap()
    biases = nc.dram_tensor("biases", [PB, NBIAS], F32, kind="ExternalInput").ap()
    out = nc.dram_tensor("out", [H + 1, S], F32, kind="ExternalOutput").ap()

    with tile.TileContext(nc) as tc:
        with (
            tc.tile_pool(name="const", bufs=1) as constp,
            tc.tile_pool(name="xs", bufs=CFG["xbufs"]) as xpool,
            tc.tile_pool(name="qkv", bufs=1) as qkvp,
            tc.tile_pool(name="qsm", bufs=2) as qsmp,
            tc.tile_pool(name="wt", bufs=CFG["wtbufs"]) as wtp,
            tc.tile_pool(name="fin", bufs=2) as finp,
            tc.tile_pool(name="pkv", bufs=2, space="PSUM") as pvp,
            tc.tile_pool(name="pq", bufs=2, space="PSUM") as pqp,
            tc.tile_pool(name="ps", bufs=CFG["psbufs"], space="PSUM") as psp,
            tc.tile_pool(name="pav", bufs=1, space="PSUM") as pavp,
        ):
            # t=0 preamble: memset a scratch line, then (a) a dummy exp on ACT
            # so the 1283ns activation-table load runs during the DMA wait,
            # (b) a chain of tiny PE matmuls so the PE p-state ramp (2.4GHz
            # only after 3us of continuous busy) burns off before real work.
            scratch = constp.tile([1, PB], BF16)
            nc.vector.memset(scratch[:, 0:H], 0)
            p_warm = pavp.tile([1, H], F32, tag="pav")
            for i in range(CFG["warmups"]):
                nc.tensor.matmul(
                    p_warm[:],
                    scratch[0:1, 0:1],
                    scratch[0:1, 0:H],
                    start=(i == 0),
                    stop=(i == CFG["warmups"] - 1),
                )

            # wqkv split in two on the scalar queue (nothing else uses it at
            # startup): the e=0/1 slice unblocks the first projection matmul
            # earlier than one big transfer.
            wqkv_sb = constp.tile([PB, ET, WQP], BF16)
            nc.scalar.dma_start(
                wqkv_sb[:, 0:2, :],
                wqkv[0 : 2 * PB, :].rearrange("(t p) m -> p t m", p=PB),
            )
            nc.scalar.dma_start(
                wqkv_sb[:, 2:ET, :],
                wqkv[2 * PB :, :].rearrange("(t p) m -> p t m", p=PB),
            )
            cs = constp.tile([PB, NCONST], BF16)
            bs = constp.tile([PB, NBIAS], F32)

            id128_ap = cs[:, C_ID : C_ID + PB]
            idv_ap = cs[H:PB, C_ID + H : C_ID + PB]  # eye(64) at partitions 64:128
            tri_ap = cs[:, C_TRI : C_TRI + PB]
            kb_ap = bs[0:H, CB_K : CB_K + 1]
            vb_ap = bs[H:PB, CB_V : CB_V + 1]
            qb_ap = bs[0:H, CB_Q : CB_Q + 1]
            ones_ap = cs[:, C_ONES : C_ONES + 1]

            qT = qkvp.tile([H, S], BF16)  # q/8 h-major
            kT = qkvp.tile([H, S], BF16)  # k h-major
            vTh = qkvp.tile([PB, S], BF16)  # v h-major at partitions 64:128
            vsb = qkvp.tile([PB, KT, H + 1], BF16)  # v k-major + ones col

            def load_consts():
                # emitted after chunk 0's x tiles so the pool queue's first
                # deliveries are the tiles the first accumulation needs
                getattr(nc, CFG["cs_q"]).dma_start(cs[:], consts[:])
                getattr(nc, CFG["cs_q"]).dma_start(bs[:], biases[:])
                nc.vector.tensor_copy(
                    vsb[:, :, H : H + 1],
                    ones_ap[:, 0:1, None].to_broadcast((PB, KT, 1)),
                )
                # dummy exp: loads the ACT Exp table during the DMA wait
                nc.scalar.activation(scratch[0:1, H : H + 1], scratch[0:1, 0:1], AF.Exp)

            def proj_main(c):
                # DMA + accumulating matmuls, then the epilogue, all emitted
                # with yields so the previous chunk's attention interleaves:
                # the in-order PE queue then has AV work between the epilogue
                # transposes (which wait on the DVE qsm copy).
                qs = slice(c * QB, (c + 1) * QB)
                p_kv = pvp.tile([PB, QB], F32, tag="pkv")
                # q s-major; full-bank tile so its start's zero region (the
                # whole 2KB PSUM bank) can't clobber a co-resident tile
                p_q = pqp.tile([PB, QB], F32, tag="pq")
                for e in range(ET):
                    xt = xpool.tile([PB, QB], BF16, tag="xt")
                    if c == 0:
                        # scalar (busy with wqkv + the act-table preload) gets
                        # only one startup tile; sync/gpsimd carry the rest
                        dma_eng = {
                            0: nc.sync, 1: nc.scalar, 2: getattr(nc, CFG["dma2"]),
                            3: nc.sync, 4: getattr(nc, CFG["dma2"]), 5: nc.sync,
                            6: getattr(nc, CFG["dma2"]), 7: nc.sync,
                        }[e]
                    else:
                        dma_eng = nc.sync if e % 2 == 0 else getattr(nc, CFG["dma2"])
                    dma_eng.dma_start(xt[:], xT[e * PB : (e + 1) * PB, qs])
                    nc.tensor.matmul(
                        p_kv[:],
                        wqkv_sb[:, e, 0:PB],
                        xt[:],
                        start=(e == 0),
                        stop=(e == ET - 1),
                    )
                    # q s-major: x block as stationary, Wq.T as moving. Four
                    # accumulation groups share one PSUM bank: only the very
                    # first matmul may set start (start zeroes the whole
                    # bank); groups 1-3's first writes land on still-pending
                    # zero bytes and store rather than accumulate.
                    for j in range(DIAG):
                        nc.tensor.matmul(
                            p_q[:, j * H : (j + 1) * H],
                            xt[:, j * PB : (j + 1) * PB],
                            wqkv_sb[:, e, 2 * H : 3 * H],
                            start=(e == 0 and j == 0),
                            stop=(e == ET - 1 and j == DIAG - 1),
                            skip_group_check=True,
                        )
                    yield
                # epilogue: all copies on DVE (ACT is saturated by exps), the
                # qsm halves first since they gate the PE transposes
                qsm = qsmp.tile([PB, DIAG * H], BF16, tag="qsm")
                nc.vector.tensor_copy(qsm[:, 0 : 2 * H], p_q[:, 0 : 2 * H])
                nc.vector.tensor_copy(qsm[:, 2 * H : 4 * H], p_q[:, 2 * H : 4 * H])
                nc.vector.tensor_scalar(
                    kT[:, qs], p_kv[0:H, :], kb_ap, None, ADD, mybir.AluOpType.bypass
                )
                nc.vector.tensor_scalar(
                    vTh[H:PB, qs], p_kv[H:PB, :], vb_ap, None, ADD, mybir.AluOpType.bypass
                )
                yield
                # q: s-major -> PE transpose -> h-major (scale+bias on DVE)
                for j in range(DIAG):
                    p_qt = pqp.tile([H, PB], BF16, tag="pq")
                    nc.tensor.transpose(p_qt[:], qsm[:, j * H : (j + 1) * H], id128_ap)
                    nc.vector.tensor_scalar(
                        qT[:, c * QB + j * PB : c * QB + (j + 1) * PB],
                        p_qt[:],
                        0.125,
                        qb_ap,
                        MUL,
                        ADD,
                    )
                    if j % 2 == 1:
                        yield
                for t in range(DIAG):
                    m = DIAG * c + t
                    p_vt = pvp.tile([PB, H], BF16, tag="pkv")
                    nc.tensor.transpose(
                        p_vt[:], vTh[H:PB, m * PB : (m + 1) * PB], idv_ap
                    )
                    nc.vector.tensor_copy(vsb[:, m, 0:H], p_vt[:])
                    if t % 2 == 1:
                        yield

            def attn(c):
                nkt = DIAG * c + DIAG
                p_av = pavp.tile([H + 1, QB], F32, tag="pav")

                def live_lo(m):
                    i = m - DIAG * c
                    return i * PB if i > 0 else 0

                def weights_tile(m):
                    # scores -> exp -> (diagonal) causal triangle mask,
                    # live columns only
                    lo = live_lo(m)
                    p_s = psp.tile([PB, QB], F32, tag="ps")
                    nc.tensor.matmul(
                        p_s[:, lo:QB],
                        kT[:, m * PB : (m + 1) * PB],
                        qT[:, c * QB + lo : (c + 1) * QB],
                        start=True,
                        stop=True,
                    )
                    w = wtp.tile([PB, QB], BF16, tag="w")
                    nc.scalar.activation(w[:, lo:QB], p_s[:, lo:QB], AF.Exp)
                    i = m - DIAG * c
                    if i >= 0:
                        nc.vector.tensor_tensor(
                            w[:, lo : lo + PB], w[:, lo : lo + PB], tri_ap, MUL
                        )
                    return w

                L = CFG["lookahead"]
                if c == QC - 1 and CFG["diag_first"]:
                    # final chunk: diagonals first so the drain of the last
                    # (unpipelined) m-steps has no exp->mask->AV chain
                    order = list(range(DIAG * c, nkt)) + list(range(0, DIAG * c))
                else:
                    order = list(range(nkt))
                ws = {m: weights_tile(m) for m in order[: min(L, nkt)]}
                yield
                for idx, m in enumerate(order):
                    if idx + L < nkt:
                        ws[order[idx + L]] = weights_tile(order[idx + L])
                    lo = live_lo(m)
                    nc.tensor.matmul(
                        p_av[:, lo:QB],
                        vsb[:, m, :],
                        ws.pop(m)[:, lo:QB],
                        start=(idx == 0),
                        stop=(idx == nkt - 1),
                    )
                    yield
                # unnormalized output + denominator row; division happens on
                # the host as part of unsharding. The last chunk goes out in
                # column halves so half 2's copy overlaps half 1's DMA.
                osb = finp.tile([H + 1, QB], F32, tag="osb")
                if c == QC - 1:
                    hw_ = QB // 2
                    for hh in range(2):
                        cols = slice(hh * hw_, (hh + 1) * hw_)
                        nc.vector.tensor_copy(osb[:, cols], p_av[:, cols])
                        oq = nc.sync if hh == 0 else nc.scalar
                        oq.dma_start(
                            out[:, c * QB + hh * hw_ : c * QB + (hh + 1) * hw_],
                            osb[:, cols],
                        )
                        yield
                else:
                    nc.vector.tensor_copy(osb[:], p_av[:])
                    yield
                    nc.sync.dma_start(out[:, c * QB : (c + 1) * QB], osb[:])
                    yield

            # interleaved emission: proj_main(c) alternates with attn(c-1) so
            # the in-order engine queues see attention work during DMA waits;
            # each projection epilogue is emitted after that attention so no
            # exp/mask queues behind an epilogue copy still waiting on DMA.
            g0 = proj_main(0)
            for _ in range(2):
                next(g0)  # chunk 0's first x tiles lead the DMA queues
            load_consts()
            _interleave(g0)
            for c in range(1, QC):
                _interleave(proj_main(c), attn(c - 1))
            _interleave(attn(QC - 1))

    nc.compile()
    return nc


def _host_inputs(x, Wq, bq, Wk, bk, Wv, bv):
    bf16 = ml_dtypes.bfloat16
    x = np.asarray(x, np.float32)
    Wq, bq = np.asarray(Wq, np.float32), np.asarray(bq, np.float32)
    Wk, bk = np.asarray(Wk, np.float32), np.asarray(bk, np.float32)
    Wv, bv = np.asarray(Wv, np.float32), np.asarray(bv, np.float32)

    wqkv = np.zeros((E, WQP), np.float32)  # padded to 512B rows for full DMA rate
    wqkv[:, 0 : 3 * H] = np.concatenate([Wk.T, Wv.T, Wq.T], axis=1)
    wqkv = wqkv.astype(bf16)

    cs = np.zeros((PB, NCONST), np.float32)
    cs[:, C_ID : C_ID + PB] = np.eye(PB, dtype=np.float32)
    jj = np.arange(PB, dtype=np.int64)[None, :]
    pp = np.arange(PB, dtype=np.int64)[:, None]
    cs[:, C_TRI : C_TRI + PB] = (jj >= pp).astype(np.float32)
    cs[:, C_ONES] = 1.0
    cs = cs.astype(bf16)

    bsc = np.zeros((PB, NBIAS), np.float32)
    bsc[:H, CB_K] = bk
    bsc[H:PB, CB_V] = bv
    bsc[:H, CB_Q] = bq * 0.125

    shared = {"wqkv": wqkv, "consts": cs, "biases": bsc}
    in_maps = []
    for b in range(B):
        m = dict(shared)
        m["xT"] = np.ascontiguousarray(x[b].T).astype(bf16)
        in_maps.append(m)
    return in_maps


def get_nc():
    if "nc" not in _CACHE:
        _CACHE["nc"] = _build_nc()
    return _CACHE["nc"]


def kernel(x, Wq, bq, Wk, bk, Wv, bv):
    nc = get_nc()
    in_maps = _host_inputs(x, Wq, bq, Wk, bk, Wv, bv)
    results = bass2jax.run_bass_via_pjrt(nc, in_maps, n_cores=NCORES)
    out = np.empty((B, S, H), np.float32)
    for b in range(B):
        o = results[b]["out"]
        out[b] = (o[:H] / o[H : H + 1]).T
    return out


# revision 26
# speedup vs baseline: 1.0414x; 1.0414x over previous
"""Single-head causal attention on 8 Trainium2 NeuronCores.

Problem: x:[8,2048,1024], Wq/Wk/Wv:[64,1024], bq/bk/bv:[64]
  q,k,v = x@W*.T + b*;  out = softmax(causal(q@k.T)/sqrt(64)) @ v

Sharding: batch dim (8) across the 8 cores - fully data-parallel, no
collectives. Each core computes one batch's attention head.

v2 design (all matmuls bf16, 1 cycle/row on the PE at any free size):
  - x and the packed projection weights are converted to bf16 on the host:
    halves HBM traffic and frees the fp32r >=256-free-dim constraint.
  - k/v projection stays h-major ([Wk|Wv] packed stationary, x moving:
    full 128-wide output). q projection runs s-major: x tiles double as
    stationary [128e,128s] blocks, Wq.T [128e,64] is the moving operand -
    64 rows per (e,s) pair instead of 512 per (e,chunk), 2x fewer PE rows
    for q. q then transposes back to h-major on the PE (bf16 identity).
  - causal dead-column skipping: for diagonal k-tiles only columns
    [128i, 512) are live; scores/exp/AV all operate on the live slice
    (saves ~2.5us PE + ~2.5us ACT). Only the [128,128] triangle block
    needs the 0/1 mask multiply (DVE, bf16).
  - ACT runs exp almost exclusively (the bottleneck engine in v1 at 82%);
    q/k epilogue copies live on DVE, v on ACT, outputs DMA straight from
    PSUM (no SBUF staging).
  - softmax without max-subtraction (scores/8 ~ N(0,1); exp safe in f32),
    denominator = ones-row appended to V, division on the host.
  - emission interleaves proj(c+1) with attn(c); the last chunk processes
    its (cheap, live-sliced) diagonal tiles first so the drain is fed by
    exps computed well in advance.
"""

import numpy as np
import ml_dtypes

import concourse.bacc as bacc
import concourse.mybir as mybir
import concourse.tile as tile
from concourse import bass2jax

B, S, E, H = 8, 2048, 1024, 64
NCORES = 8
PB = 128  # partition block / k-tile size
QB = 512  # q-chunk (matmul moving free dim)
ET = E // PB  # e-tiles per contraction
QC = S // QB  # q-chunks
KT = S // PB  # k-tiles
DIAG = QB // PB  # diagonal k-tiles per q-chunk

# packed bf16 constants layout: columns of the [128, NCONST] "consts" input
C_ID = 0  # [*, 0:128]    identity 128x128
C_TRI = 128  # [*, 128:256]  causal triangle: (p, j) = 1 iff j >= p
C_ONES = 256  # [*, 256:257]  ones
NCONST = 257
WQP = 2 * PB  # wqkv padded row length (bf16 512B descriptors, full DMA rate)
# f32 bias constants: columns of the [128, NBIAS] "biases" input
CB_K = 0  # k bias (rows 0:64)
CB_V = 1  # v bias (rows 64:128)
CB_Q = 2  # q bias / 8 (rows 0:64)
NBIAS = 3

F32 = mybir.dt.float32
BF16 = mybir.dt.bfloat16
AF = mybir.ActivationFunctionType
MUL = mybir.AluOpType.mult
ADD = mybir.AluOpType.add

_CACHE: dict = {}

# schedule/buffering knobs (sweepable)
CFG = {
    "lookahead": 7,
    "xbufs": 16,
    "wtbufs": 10,
    "psbufs": 3,
    "dma2": "gpsimd",  # second x-stream queue
    "cs_q": "gpsimd",
    "diag_first": True,
    "warmups": 58,  # PE p-state warmup matmuls (64-free) during startup DMA
}


def _interleave(*gens):
    """Drive generators round-robin; the first (proj) gets two steps per turn."""
    alive = list(gens)
    steps = {id(g): (2 if i == 0 and len(gens) > 1 else 1) for i, g in enumerate(gens)}
    while alive:
        for g in list(alive):
            for _ in range(steps[id(g)]):
                try:
                    next(g)
                except StopIteration:
                    alive.remove(g)
                    break


def _build_nc():
    nc = bacc.Bacc("TRN2", target_bir_lowering=False, debug=False)
    xT = nc.dram_tensor("xT", [E, S], BF16, kind="ExternalInput").ap()
    # cols 0:64 Wk.T, 64:128 Wv.T, 128:192 Wq.T, 192:256 zero padding
    wqkv = nc.dram_tensor("wqkv", [E, WQP], BF16, kind="ExternalInput").ap()
    consts = nc.dram_tensor("consts", [PB, NCONST], BF16, kind="ExternalInput").ap()
    biases = nc.dram_tensor("biases", [PB, NBIAS], F32, kind="ExternalInput").ap()
    out = nc.dram_tensor("out", [H + 1, S], F32, kind="ExternalOutput").ap()

    with tile.TileContext(nc) as tc:
        with (
            tc.tile_pool(name="const", bufs=1) as constp,
            tc.tile_pool(name="xs", bufs=CFG["xbufs"]) as xpool,
            tc.tile_pool(name="qkv", bufs=1) as qkvp,
            tc.tile_pool(name="qsm", bufs=2) as qsmp,
            tc.tile_pool(name="wt", bufs=CFG["wtbufs"]) as wtp,
            tc.tile_pool(name="fin", bufs=2) as finp,
            tc.tile_pool(name="pkv", bufs=2, space="PSUM") as pvp,
            tc.tile_pool(name="pq", bufs=2, space="PSUM") as pqp,
            tc.tile_pool(name="ps", bufs=CFG["psbufs"], space="PSUM") as psp,
            tc.tile_pool(name="pav", bufs=1, space="PSUM") as pavp,
        ):
            # t=0 preamble: memset a scratch line, then (a) a dummy exp on ACT
            # so the 1283ns activation-table load runs during the DMA wait,
            # (b) a chain of tiny PE matmuls so the PE p-state ramp (2.4GHz
            # only after 3us of continuous busy) burns off before real work.
            scratch = constp.tile([1, PB], BF16)
            nc.vector.memset(scratch[:, 0:H], 0)
            p_warm = pavp.tile([1, H], F32, tag="pav")
            for i in range(CFG["warmups"]):
                nc.tensor.matmul(
                    p_warm[:],
                    scratch[0:1, 0:1],
                    scratch[0:1, 0:H],
                    start=(i == 0),
                    stop=(i == CFG["warmups"] - 1),
                )

            # wqkv leads the sync queue in three slices (e0 / e1-3 / e4-7) so
            # the first slice's transfer beats the x tiles into the shared
            # DMA-engine FIFO and the later slices pipeline under them.
            wqkv_sb = constp.tile([PB, ET, WQP], BF16)
            for lo, hi in ((0, 1), (1, 4), (4, ET)):
                nc.sync.dma_start(
                    wqkv_sb[:, lo:hi, :],
                    wqkv[lo * PB : hi * PB, :].rearrange("(t p) m -> p t m", p=PB),
                )
            cs = constp.tile([PB, NCONST], BF16)
            bs = constp.tile([PB, NBIAS], F32)

            id128_ap = cs[:, C_ID : C_ID + PB]
            idv_ap = cs[H:PB, C_ID + H : C_ID + PB]  # eye(64) at partitions 64:128
            tri_ap = cs[:, C_TRI : C_TRI + PB]
            kb_ap = bs[0:H, CB_K : CB_K + 1]
            vb_ap = bs[H:PB, CB_V : CB_V + 1]
            qb_ap = bs[0:H, CB_Q : CB_Q + 1]
            ones_ap = cs[:, C_ONES : C_ONES + 1]

            qT = qkvp.tile([H, S], BF16)  # q/8 h-major
            kT = qkvp.tile([H, S], BF16)  # k h-major
            vTh = qkvp.tile([PB, S], BF16)  # v h-major at partitions 64:128
            vsb = qkvp.tile([PB, KT, H + 1], BF16)  # v k-major + ones col

            def load_consts():
                # emitted after chunk 0's x tiles so the pool queue's first
                # deliveries are the tiles the first accumulation needs
                getattr(nc, CFG["cs_q"]).dma_start(cs[:], consts[:])
                getattr(nc, CFG["cs_q"]).dma_start(bs[:], biases[:])
                nc.vector.tensor_copy(
                    vsb[:, :, H : H + 1],
                    ones_ap[:, 0:1, None].to_broadcast((PB, KT, 1)),
                )
                # dummy exp: loads the ACT Exp table during the DMA wait
                nc.scalar.activation(scratch[0:1, H : H + 1], scratch[0:1, 0:1], AF.Exp)

            def proj_main(c):
                # DMA + accumulating matmuls, then the epilogue, all emitted
                # with yields so the previous chunk's attention interleaves:
                # the in-order PE queue then has AV work between the epilogue
                # transposes (which wait on the DVE qsm copy).
                qs = slice(c * QB, (c + 1) * QB)
                p_kv = pvp.tile([PB, QB], F32, tag="pkv")
                # q s-major; full-bank tile so its start's zero region (the
                # whole 2KB PSUM bank) can't clobber a co-resident tile
                p_q = pqp.tile([PB, QB], F32, tag="pq")
                for e in range(ET):
                    xt = xpool.tile([PB, QB], BF16, tag="xt")
                    if c == 0:
                        # sync is busy with the wqkv slices: x startup tiles
                        # ride scalar/gpsimd first, sync late
                        dma_eng = {
                            0: nc.scalar, 1: nc.scalar, 2: getattr(nc, CFG["dma2"]),
                            3: getattr(nc, CFG["dma2"]), 4: nc.scalar, 5: nc.sync,
                            6: getattr(nc, CFG["dma2"]), 7: nc.sync,
                        }[e]
                    else:
                        dma_eng = nc.sync if e % 2 == 0 else getattr(nc, CFG["dma2"])
                    dma_eng.dma_start(xt[:], xT[e * PB : (e + 1) * PB, qs])
                    nc.tensor.matmul(
                        p_kv[:],
                        wqkv_sb[:, e, 0:PB],
                        xt[:],
                        start=(e == 0),
                        stop=(e == ET - 1),
                    )
                    # q s-major: x block as stationary, Wq.T as moving. Four
                    # accumulation groups share one PSUM bank: only the very
                    # first matmul may set start (start zeroes the whole
                    # bank); groups 1-3's first writes land on still-pending
                    # zero bytes and store rather than accumulate.
                    for j in range(DIAG):
                        nc.tensor.matmul(
                            p_q[:, j * H : (j + 1) * H],
                            xt[:, j * PB : (j + 1) * PB],
                            wqkv_sb[:, e, 2 * H : 3 * H],
                            start=(e == 0 and j == 0),
                            stop=(e == ET - 1 and j == DIAG - 1),
                            skip_group_check=True,
                        )
                    yield
                # epilogue: all copies on DVE (ACT is saturated by exps).
                # DVE order: qsm halves (gate the PE transposes), k, then the
                # qT scalars; v/vsb follow — the next chunk's scores need
                # q and k first.
                qsm = qsmp.tile([PB, DIAG * H], BF16, tag="qsm")
                nc.vector.tensor_copy(qsm[:, 0 : 2 * H], p_q[:, 0 : 2 * H])
                nc.vector.tensor_copy(qsm[:, 2 * H : 4 * H], p_q[:, 2 * H : 4 * H])
                nc.vector.tensor_scalar(
                    kT[:, qs], p_kv[0:H, :], kb_ap, None, ADD, mybir.AluOpType.bypass
                )
                yield
                # q: s-major -> PE transpose -> h-major (scale+bias on DVE)
                for j in range(DIAG):
                    p_qt = pqp.tile([H, PB], BF16, tag="pq")
                    nc.tensor.transpose(p_qt[:], qsm[:, j * H : (j + 1) * H], id128_ap)
                    nc.vector.tensor_scalar(
                        qT[:, c * QB + j * PB : c * QB + (j + 1) * PB],
                        p_qt[:],
                        0.125,
                        qb_ap,
                        MUL,
                        ADD,
                    )
                    if j % 2 == 1:
                        yield
                nc.vector.tensor_scalar(
                    vTh[H:PB, qs], p_kv[H:PB, :], vb_ap, None, ADD, mybir.AluOpType.bypass
                )
                for t in range(DIAG):
                    m = DIAG * c + t
                    p_vt = pvp.tile([PB, H], BF16, tag="pkv")
                    nc.tensor.transpose(
                        p_vt[:], vTh[H:PB, m * PB : (m + 1) * PB], idv_ap
                    )
                    nc.vector.tensor_copy(vsb[:, m, 0:H], p_vt[:])
                    if t % 2 == 1:
                        yield

            def attn(c):
                nkt = DIAG * c + DIAG
                p_av = pavp.tile([H + 1, QB], F32, tag="pav")

                def live_lo(m):
                    i = m - DIAG * c
                    return i * PB if i > 0 else 0

                def weights_tile(m):
                    # scores -> exp -> (diagonal) causal triangle mask,
                    # live columns only
                    lo = live_lo(m)
                    p_s = psp.tile([PB, QB], F32, tag="ps")
                    nc.tensor.matmul(
                        p_s[:, lo:QB],
                        kT[:, m * PB : (m + 1) * PB],
                        qT[:, c * QB + lo : (c + 1) * QB],
                        start=True,
                        stop=True,
                    )
                    w = wtp.tile([PB, QB], BF16, tag="w")
                    nc.scalar.activation(w[:, lo:QB], p_s[:, lo:QB], AF.Exp)
                    i = m - DIAG * c
                    if i >= 0:
                        nc.vector.tensor_tensor(
                            w[:, lo : lo + PB], w[:, lo : lo + PB], tri_ap, MUL
                        )
                    return w

                L = CFG["lookahead"]
                if c == QC - 1 and CFG["diag_first"]:
                    # final chunk: diagonals first so the drain of the last
                    # (unpipelined) m-steps has no exp->mask->AV chain
                    order = list(range(DIAG * c, nkt)) + list(range(0, DIAG * c))
                else:
                    order = list(range(nkt))
                ws = {m: weights_tile(m) for m in order[: min(L, nkt)]}
                yield
                for idx, m in enumerate(order):
                    if idx + L < nkt:
                        ws[order[idx + L]] = weights_tile(order[idx + L])
                    lo = live_lo(m)
                    nc.tensor.matmul(
                        p_av[:, lo:QB],
                        vsb[:, m, :],
                        ws.pop(m)[:, lo:QB],
                        start=(idx == 0),
                        stop=(idx == nkt - 1),
                    )
                    yield
                # unnormalized output + denominator row; division happens on
                # the host as part of unsharding. The last chunk goes out in
                # column halves so half 2's copy overlaps half 1's DMA.
                osb = finp.tile([H + 1, QB], F32, tag="osb")
                if c == QC - 1:
                    hw_ = QB // 2
                    for hh in range(2):
                        cols = slice(hh * hw_, (hh + 1) * hw_)
                        nc.vector.tensor_copy(osb[:, cols], p_av[:, cols])
                        oq = nc.sync if hh == 0 else nc.scalar
                        oq.dma_start(
                            out[:, c * QB + hh * hw_ : c * QB + (hh + 1) * hw_],
                            osb[:, cols],
                        )
                        yield
                else:
                    nc.vector.tensor_copy(osb[:], p_av[:])
                    yield
                    nc.sync.dma_start(out[:, c * QB : (c + 1) * QB], osb[:])
                    yield

            # interleaved emission: proj_main(c) alternates with attn(c-1) so
            # the in-order engine queues see attention work during DMA waits;
            # each projection epilogue is emitted after that attention so no
            # exp/mask queues behind an epilogue copy still waiting on DMA.
            g0 = proj_main(0)
            for _ in range(2):
                next(g0)  # chunk 0's first x tiles lead the DMA queues
            load_consts()
            _interleave(g0)
            for c in range(1, QC):
                _interleave(proj_main(c), attn(c - 1))
            _interleave(attn(QC - 1))

    nc.compile()
    return nc


def _host_inputs(x, Wq, bq, Wk, bk, Wv, bv):
    bf16 = ml_dtypes.bfloat16
    x = np.asarray(x, np.float32)
    Wq, bq = np.asarray(Wq, np.float32), np.asarray(bq, np.float32)
    Wk, bk = np.asarray(Wk, np.float32), np.asarray(bk, np.float32)
    Wv, bv = np.asarray(Wv, np.float32), np.asarray(bv, np.float32)

    wqkv = np.zeros((E, WQP), np.float32)  # padded to 512B rows for full DMA rate
    wqkv[:, 0 : 3 * H] = np.concatenate([Wk.T, Wv.T, Wq.T], axis=1)
    wqkv = wqkv.astype(bf16)

    cs = np.zeros((PB, NCONST), np.float32)
    cs[:, C_ID : C_ID + PB] = np.eye(PB, dtype=np.float32)
    jj = np.arange(PB, dtype=np.int64)[None, :]
    pp = np.arange(PB, dtype=np.int64)[:, None]
    cs[:, C_TRI : C_TRI + PB] = (jj >= pp).astype(np.float32)
    cs[:, C_ONES] = 1.0
    cs = cs.astype(bf16)

    bsc = np.zeros((PB, NBIAS), np.float32)
    bsc[:H, CB_K] = bk
    bsc[H:PB, CB_V] = bv
    bsc[:H, CB_Q] = bq * 0.125

    shared = {"wqkv": wqkv, "consts": cs, "biases": bsc}
    in_maps = []
    for b in range(B):
        m = dict(shared)
        m["xT"] = np.ascontiguousarray(x[b].T).astype(bf16)
        in_maps.append(m)
    return in_maps


def get_nc():
    if "nc" not in _CACHE:
        _CACHE["nc"] = _build_nc()
    return _CACHE["nc"]


def kernel(x, Wq, bq, Wk, bk, Wv, bv):
    nc = get_nc()
    in_maps = _host_inputs(x, Wq, bq, Wk, bk, Wv, bv)
    results = bass2jax.run_bass_via_pjrt(nc, in_maps, n_cores=NCORES)
    out = np.empty((B, S, H), np.float32)
    for b in range(B):
        o = results[b]["out"]
        out[b] = (o[:H] / o[H : H + 1]).T
    return out


# revision 31
# speedup vs baseline: 1.2657x; 1.2154x over previous
"""Single-head causal attention on 8 Trainium2 NeuronCores.

Problem: x:[8,2048,1024], Wq/Wk/Wv:[64,1024], bq/bk/bv:[64]
  q,k,v = x@W*.T + b*;  out = softmax(causal(q@k.T)/sqrt(64)) @ v

Sharding: batch dim (8) across the 8 cores - fully data-parallel, no
collectives. Each core computes one batch's attention head.

v2 design (all matmuls bf16, 1 cycle/row on the PE at any free size):
  - x and the packed projection weights are converted to bf16 on the host:
    halves HBM traffic and frees the fp32r >=256-free-dim constraint.
  - k/v projection stays h-major ([Wk|Wv] packed stationary, x moving:
    full 128-wide output). q projection runs s-major: x tiles double as
    stationary [128e,128s] blocks, Wq.T [128e,64] is the moving operand -
    64 rows per (e,s) pair instead of 512 per (e,chunk), 2x fewer PE rows
    for q. q then transposes back to h-major on the PE (bf16 identity).
  - causal dead-column skipping: for diagonal k-tiles only columns
    [128i, 512) are live; scores/exp/AV all operate on the live slice
    (saves ~2.5us PE + ~2.5us ACT). Only the [128,128] triangle block
    needs the 0/1 mask multiply (DVE, bf16).
  - ACT runs exp almost exclusively (the bottleneck engine in v1 at 82%);
    q/k epilogue copies live on DVE, v on ACT, outputs DMA straight from
    PSUM (no SBUF staging).
  - softmax without max-subtraction (scores/8 ~ N(0,1); exp safe in f32),
    denominator = ones-row appended to V, division on the host.
  - emission interleaves proj(c+1) with attn(c); the last chunk processes
    its (cheap, live-sliced) diagonal tiles first so the drain is fed by
    exps computed well in advance.
"""

import numpy as np
import ml_dtypes

import concourse.bacc as bacc
import concourse.mybir as mybir
import concourse.tile as tile
from concourse import bass2jax

B, S, E, H = 8, 2048, 1024, 64
NCORES = 8
PB = 128  # partition block / k-tile size
QB = 512  # q-chunk (matmul moving free dim)
ET = E // PB  # e-tiles per contraction
QC = S // QB  # q-chunks
KT = S // PB  # k-tiles
DIAG = QB // PB  # diagonal k-tiles per q-chunk

# packed bf16 constants layout: columns of the [128, NCONST] "consts" input
C_TRI = 0  # [*, 0:128]    causal triangle: (p, j) = 1 iff j >= p
C_ONES = 128  # [*, 128:129]  ones
C_BVR = 129  # [*, 129:193]  v bias replicated on every partition row
NCONST = 193
WQP = 2 * PB  # wqkv padded row length (bf16 512B descriptors, full DMA rate)
# f32 bias constants: columns of the [128, NBIAS] "biases" input
CB_KQ = 0  # k bias / 8 (rows 0:64), q bias (rows 64:128)
NBIAS = 1

F32 = mybir.dt.float32
BF16 = mybir.dt.bfloat16
AF = mybir.ActivationFunctionType
MUL = mybir.AluOpType.mult
ADD = mybir.AluOpType.add

_CACHE: dict = {}

# schedule/buffering knobs (sweepable)
CFG = {
    "lookahead": 7,
    "xbufs": 16,
    "wtbufs": 10,
    "psbufs": 3,
    "dma2": "gpsimd",  # second x-stream queue
    "cs_q": "gpsimd",
    "diag_first": True,
    "warmups": 58,  # PE p-state warmup matmuls (64-free) during startup DMA
}


def _interleave(*gens):
    """Drive generators round-robin; the first (proj) gets two steps per turn."""
    alive = list(gens)
    steps = {id(g): (2 if i == 0 and len(gens) > 1 else 1) for i, g in enumerate(gens)}
    while alive:
        for g in list(alive):
            for _ in range(steps[id(g)]):
                try:
                    next(g)
                except StopIteration:
                    alive.remove(g)
                    break


def _build_nc():
    nc = bacc.Bacc("TRN2", target_bir_lowering=False, debug=False)
    xT = nc.dram_tensor("xT", [E, S], BF16, kind="ExternalInput").ap()
    # cols 0:64 Wk.T, 64:128 Wv.T, 128:192 Wq.T, 192:256 zero padding
    wqkv = nc.dram_tensor("wqkv", [E, WQP], BF16, kind="ExternalInput").ap()
    consts = nc.dram_tensor("consts", [PB, NCONST], BF16, kind="ExternalInput").ap()
    biases = nc.dram_tensor("biases", [PB, NBIAS], F32, kind="ExternalInput").ap()
    out = nc.dram_tensor("out", [H + 1, S], F32, kind="ExternalOutput").ap()

    with tile.TileContext(nc) as tc:
        with (
            tc.tile_pool(name="const", bufs=1) as constp,
            tc.tile_pool(name="xs", bufs=CFG["xbufs"]) as xpool,
            tc.tile_pool(name="qkv", bufs=1) as qkvp,
            tc.tile_pool(name="wt", bufs=CFG["wtbufs"]) as wtp,
            tc.tile_pool(name="fin", bufs=2) as finp,
            tc.tile_pool(name="pkq", bufs=2, space="PSUM") as pvp,
            tc.tile_pool(name="pv", bufs=2, space="PSUM") as pqp,
            tc.tile_pool(name="ps", bufs=CFG["psbufs"], space="PSUM") as psp,
            tc.tile_pool(name="pav", bufs=1, space="PSUM") as pavp,
        ):
            # t=0 preamble: memset a scratch line, then (a) a dummy exp on ACT
            # so the 1283ns activation-table load runs during the DMA wait,
            # (b) a chain of tiny PE matmuls so the PE p-state ramp (2.4GHz
            # only after 3us of continuous busy) burns off before real work.
            scratch = constp.tile([1, PB], BF16)
            nc.vector.memset(scratch[:, 0:H], 0)
            p_warm = pavp.tile([1, H], F32, tag="pav")
            for i in range(CFG["warmups"]):
                nc.tensor.matmul(
                    p_warm[:],
                    scratch[0:1, 0:1],
                    scratch[0:1, 0:H],
                    start=(i == 0),
                    stop=(i == CFG["warmups"] - 1),
                )

            # wqkv leads the sync queue in three slices (e0 / e1-3 / e4-7) so
            # the first slice's transfer beats the x tiles into the shared
            # DMA-engine FIFO and the later slices pipeline under them.
            wqkv_sb = constp.tile([PB, ET, WQP], BF16)
            for lo, hi in ((0, 1), (1, 4), (4, ET)):
                nc.sync.dma_start(
                    wqkv_sb[:, lo:hi, :],
                    wqkv[lo * PB : hi * PB, :].rearrange("(t p) m -> p t m", p=PB),
                )
            cs = constp.tile([PB, NCONST], BF16)
            bs = constp.tile([PB, NBIAS], F32)

            tri_ap = cs[:, C_TRI : C_TRI + PB]
            kb_ap = bs[0:H, CB_KQ : CB_KQ + 1]  # bk/8 at partitions 0:64
            qb_ap = bs[H:PB, CB_KQ : CB_KQ + 1]  # bq at partitions 64:128
            ones_ap = cs[:, C_ONES : C_ONES + 1]
            bvr_ap = cs[:, C_BVR : C_BVR + H]  # bv replicated per partition

            qT = qkvp.tile([H, S], BF16)  # q h-major
            kT = qkvp.tile([H, S], BF16)  # (k + bk)/8 h-major
            vsb = qkvp.tile([PB, KT, H + 1], BF16)  # v k-major + ones col

            def load_consts():
                # emitted after chunk 0's x tiles so the pool queue's first
                # deliveries are the tiles the first accumulation needs
                getattr(nc, CFG["cs_q"]).dma_start(cs[:], consts[:])
                getattr(nc, CFG["cs_q"]).dma_start(bs[:], biases[:])
                nc.vector.tensor_copy(
                    vsb[:, :, H : H + 1],
                    ones_ap[:, 0:1, None].to_broadcast((PB, KT, 1)),
                )
                # dummy exp: loads the ACT Exp table during the DMA wait
                nc.scalar.activation(scratch[0:1, H : H + 1], scratch[0:1, 0:1], AF.Exp)

            def proj_main(c):
                # [Wk|Wq] packed h-major pass (x moving) + v s-major pass
                # (x blocks stationary, Wv.T moving, output lands k-major =
                # the vsb layout directly). No PE transposes anywhere.
                qs = slice(c * QB, (c + 1) * QB)
                p_kq = pvp.tile([PB, QB], F32, tag="pkq")
                # v s-major; full-bank tile so its start's zero region (the
                # whole 2KB PSUM bank) can't clobber a co-resident tile
                p_v = pqp.tile([PB, QB], F32, tag="pv")
                for e in range(ET):
                    xt = xpool.tile([PB, QB], BF16, tag="xt")
                    if c == 0:
                        # sync is busy with the wqkv slices: x startup tiles
                        # ride scalar/gpsimd first, sync late
                        dma_eng = {
                            0: nc.scalar, 1: nc.scalar, 2: getattr(nc, CFG["dma2"]),
                            3: getattr(nc, CFG["dma2"]), 4: nc.scalar, 5: nc.sync,
                            6: getattr(nc, CFG["dma2"]), 7: nc.sync,
                        }[e]
                    else:
                        dma_eng = nc.sync if e % 2 == 0 else getattr(nc, CFG["dma2"])
                    dma_eng.dma_start(xt[:], xT[e * PB : (e + 1) * PB, qs])
                    nc.tensor.matmul(
                        p_kq[:],
                        wqkv_sb[:, e, 0:PB],
                        xt[:],
                        start=(e == 0),
                        stop=(e == ET - 1),
                    )
                    # v s-major: x block as stationary, Wv.T as moving. Four
                    # accumulation groups share one PSUM bank: only the very
                    # first matmul may set start (start zeroes the whole
                    # bank); groups 1-3's first writes land on still-pending
                    # zero bytes and store rather than accumulate.
                    for j in range(DIAG):
                        nc.tensor.matmul(
                            p_v[:, j * H : (j + 1) * H],
                            xt[:, j * PB : (j + 1) * PB],
                            wqkv_sb[:, e, 2 * H : 3 * H],
                            start=(e == 0 and j == 0),
                            stop=(e == ET - 1 and j == DIAG - 1),
                            skip_group_check=True,
                        )
                    yield
                # epilogue, all on DVE: k (pre-scaled by 1/8) and q gate the
                # next attention's scores; vsb blocks follow.
                nc.vector.tensor_scalar(
                    kT[:, qs], p_kq[0:H, :], 0.125, kb_ap, MUL, ADD
                )
                nc.vector.tensor_scalar(
                    qT[:, qs], p_kq[H:PB, :], qb_ap, None, ADD, mybir.AluOpType.bypass
                )
                yield
                for j in range(DIAG):
                    m = DIAG * c + j
                    nc.vector.tensor_tensor(
                        vsb[:, m, 0:H], p_v[:, j * H : (j + 1) * H], bvr_ap, ADD
                    )
                    if j % 2 == 1:
                        yield

            def attn(c):
                nkt = DIAG * c + DIAG
                p_av = pavp.tile([H + 1, QB], F32, tag="pav")

                def live_lo(m):
                    i = m - DIAG * c
                    return i * PB if i > 0 else 0

                def weights_tile(m):
                    # scores -> exp -> (diagonal) causal triangle mask,
                    # live columns only
                    lo = live_lo(m)
                    p_s = psp.tile([PB, QB], F32, tag="ps")
                    nc.tensor.matmul(
                        p_s[:, lo:QB],
                        kT[:, m * PB : (m + 1) * PB],
                        qT[:, c * QB + lo : (c + 1) * QB],
                        start=True,
                        stop=True,
                    )
                    w = wtp.tile([PB, QB], BF16, tag="w")
                    nc.scalar.activation(w[:, lo:QB], p_s[:, lo:QB], AF.Exp)
                    i = m - DIAG * c
                    if i >= 0:
                        nc.vector.tensor_tensor(
                            w[:, lo : lo + PB], w[:, lo : lo + PB], tri_ap, MUL
                        )
                    return w

                L = CFG["lookahead"]
                if c == QC - 1 and CFG["diag_first"]:
                    # final chunk: diagonals first so the drain of the last
                    # (unpipelined) m-steps has no exp->mask->AV chain
                    order = list(range(DIAG * c, nkt)) + list(range(0, DIAG * c))
                else:
                    order = list(range(nkt))
                ws = {m: weights_tile(m) for m in order[: min(L, nkt)]}
                yield
                for idx, m in enumerate(order):
                    if idx + L < nkt:
                        ws[order[idx + L]] = weights_tile(order[idx + L])
                    lo = live_lo(m)
                    nc.tensor.matmul(
                        p_av[:, lo:QB],
                        vsb[:, m, :],
                        ws.pop(m)[:, lo:QB],
                        start=(idx == 0),
                        stop=(idx == nkt - 1),
                    )
                    yield
                # unnormalized output + denominator row; division happens on
                # the host as part of unsharding. The last chunk goes out in
                # column halves so half 2's copy overlaps half 1's DMA.
                osb = finp.tile([H + 1, QB], F32, tag="osb")
                if c == QC - 1:
                    hw_ = QB // 2
                    for hh in range(2):
                        cols = slice(hh * hw_, (hh + 1) * hw_)
                        nc.vector.tensor_copy(osb[:, cols], p_av[:, cols])
                        oq = nc.sync if hh == 0 else nc.scalar
                        oq.dma_start(
                            out[:, c * QB + hh * hw_ : c * QB + (hh + 1) * hw_],
                            osb[:, cols],
                        )
                        yield
                else:
                    nc.vector.tensor_copy(osb[:], p_av[:])
                    yield
                    nc.sync.dma_start(out[:, c * QB : (c + 1) * QB], osb[:])
                    yield

            # interleaved emission: proj_main(c) alternates with attn(c-1) so
            # the in-order engine queues see attention work during DMA waits;
            # each projection epilogue is emitted after that attention so no
            # exp/mask queues behind an epilogue copy still waiting on DMA.
            g0 = proj_main(0)
            for _ in range(2):
                next(g0)  # chunk 0's first x tiles lead the DMA queues
            load_consts()
            _interleave(g0)
            for c in range(1, QC):
                _interleave(proj_main(c), attn(c - 1))
            _interleave(attn(QC - 1))

    nc.compile()
    return nc


def _host_inputs(x, Wq, bq, Wk, bk, Wv, bv):
    bf16 = ml_dtypes.bfloat16
    x = np.asarray(x, np.float32)
    Wq, bq = np.asarray(Wq, np.float32), np.asarray(bq, np.float32)
    Wk, bk = np.asarray(Wk, np.float32), np.asarray(bk, np.float32)
    Wv, bv = np.asarray(Wv, np.float32), np.asarray(bv, np.float32)

    wqkv = np.zeros((E, WQP), np.float32)  # padded to 512B rows for full DMA rate
    wqkv[:, 0 : 3 * H] = np.concatenate([Wk.T, Wq.T, Wv.T], axis=1)
    wqkv = wqkv.astype(bf16)

    cs = np.zeros((PB, NCONST), np.float32)
    jj = np.arange(PB, dtype=np.int64)[None, :]
    pp = np.arange(PB, dtype=np.int64)[:, None]
    cs[:, C_TRI : C_TRI + PB] = (jj >= pp).astype(np.float32)
    cs[:, C_ONES] = 1.0
    cs[:, C_BVR : C_BVR + H] = bv[None, :]
    cs = cs.astype(bf16)

    bsc = np.zeros((PB, NBIAS), np.float32)
    bsc[:H, CB_KQ] = bk * 0.125
    bsc[H:PB, CB_KQ] = bq

    shared = {"wqkv": wqkv, "consts": cs, "biases": bsc}
    in_maps = []
    for b in range(B):
        m = dict(shared)
        m["xT"] = np.ascontiguousarray(x[b].T).astype(bf16)
        in_maps.append(m)
    return in_maps


def get_nc():
    if "nc" not in _CACHE:
        _CACHE["nc"] = _build_nc()
    return _CACHE["nc"]


def kernel(x, Wq, bq, Wk, bk, Wv, bv):
    nc = get_nc()
    in_maps = _host_inputs(x, Wq, bq, Wk, bk, Wv, bv)
    results = bass2jax.run_bass_via_pjrt(nc, in_maps, n_cores=NCORES)
    out = np.empty((B, S, H), np.float32)
    for b in range(B):
        o = results[b]["out"]
        out[b] = (o[:H] / o[H : H + 1]).T
    return out


# revision 34
# speedup vs baseline: 1.3128x; 1.0372x over previous
"""Single-head causal attention on 8 Trainium2 NeuronCores.

Problem: x:[8,2048,1024], Wq/Wk/Wv:[64,1024], bq/bk/bv:[64]
  q,k,v = x@W*.T + b*;  out = softmax(causal(q@k.T)/sqrt(64)) @ v

Sharding: batch dim (8) across the 8 cores - fully data-parallel, no
collectives. Each core computes one batch's attention head.

v2 design (all matmuls bf16, 1 cycle/row on the PE at any free size):
  - x and the packed projection weights are converted to bf16 on the host:
    halves HBM traffic and frees the fp32r >=256-free-dim constraint.
  - k/v projection stays h-major ([Wk|Wv] packed stationary, x moving:
    full 128-wide output). q projection runs s-major: x tiles double as
    stationary [128e,128s] blocks, Wq.T [128e,64] is the moving operand -
    64 rows per (e,s) pair instead of 512 per (e,chunk), 2x fewer PE rows
    for q. q then transposes back to h-major on the PE (bf16 identity).
  - causal dead-column skipping: for diagonal k-tiles only columns
    [128i, 512) are live; scores/exp/AV all operate on the live slice
    (saves ~2.5us PE + ~2.5us ACT). Only the [128,128] triangle block
    needs the 0/1 mask multiply (DVE, bf16).
  - ACT runs exp almost exclusively (the bottleneck engine in v1 at 82%);
    q/k epilogue copies live on DVE, v on ACT, outputs DMA straight from
    PSUM (no SBUF staging).
  - softmax without max-subtraction (scores/8 ~ N(0,1); exp safe in f32),
    denominator = ones-row appended to V, division on the host.
  - emission interleaves proj(c+1) with attn(c); the last chunk processes
    its (cheap, live-sliced) diagonal tiles first so the drain is fed by
    exps computed well in advance.
"""

import numpy as np
import ml_dtypes

import concourse.bacc as bacc
import concourse.mybir as mybir
import concourse.tile as tile
from concourse import bass2jax

B, S, E, H = 8, 2048, 1024, 64
NCORES = 8
PB = 128  # partition block / k-tile size
QB = 512  # q-chunk (matmul moving free dim)
ET = E // PB  # e-tiles per contraction
QC = S // QB  # q-chunks
KT = S // PB  # k-tiles
DIAG = QB // PB  # diagonal k-tiles per q-chunk

# packed bf16 constants layout: columns of the [128, NCONST] "consts" input
C_TRI = 0  # [*, 0:128]    causal triangle: (p, j) = 1 iff j >= p
C_ONES = 128  # [*, 128:129]  ones
C_BVR = 129  # [*, 129:193]  v bias replicated on every partition row
NCONST = 193
WQP = 2 * PB  # wqkv padded row length (bf16 512B descriptors, full DMA rate)
# f32 bias constants: columns of the [128, NBIAS] "biases" input
CB_KQ = 0  # k bias / 8 (rows 0:64), q bias (rows 64:128)
NBIAS = 1

F32 = mybir.dt.float32
BF16 = mybir.dt.bfloat16
AF = mybir.ActivationFunctionType
MUL = mybir.AluOpType.mult
ADD = mybir.AluOpType.add

_CACHE: dict = {}

# schedule/buffering knobs (sweepable)
CFG = {
    "lookahead": 5,
    "xbufs": 16,
    "wtbufs": 6,
    "wt2bufs": 5,
    "psbufs": 2,  # [128, 2, 512] double-bank score tiles
    "dma2": "gpsimd",  # second x-stream queue
    "cs_q": "gpsimd",
    "diag_first": True,
    "warmups": 44,  # PE p-state warmup matmuls (64-free) during startup DMA
}


def _interleave(*gens):
    """Drive generators round-robin; the first (proj) gets two steps per turn."""
    alive = list(gens)
    steps = {id(g): (2 if i == 0 and len(gens) > 1 else 1) for i, g in enumerate(gens)}
    while alive:
        for g in list(alive):
            for _ in range(steps[id(g)]):
                try:
                    next(g)
                except StopIteration:
                    alive.remove(g)
                    break


def _build_nc():
    nc = bacc.Bacc("TRN2", target_bir_lowering=False, debug=False)
    xT = nc.dram_tensor("xT", [E, S], BF16, kind="ExternalInput").ap()
    # cols 0:64 Wk.T, 64:128 Wv.T, 128:192 Wq.T, 192:256 zero padding
    wqkv = nc.dram_tensor("wqkv", [E, WQP], BF16, kind="ExternalInput").ap()
    consts = nc.dram_tensor("consts", [PB, NCONST], BF16, kind="ExternalInput").ap()
    biases = nc.dram_tensor("biases", [PB, NBIAS], F32, kind="ExternalInput").ap()
    out = nc.dram_tensor("out", [H + 1, S], F32, kind="ExternalOutput").ap()

    with tile.TileContext(nc) as tc:
        with (
            tc.tile_pool(name="const", bufs=1) as constp,
            tc.tile_pool(name="xs", bufs=CFG["xbufs"]) as xpool,
            tc.tile_pool(name="qkv", bufs=1) as qkvp,
            tc.tile_pool(name="wt", bufs=CFG["wtbufs"]) as wtp,
            tc.tile_pool(name="wt2", bufs=CFG["wt2bufs"]) as wtp2,
            tc.tile_pool(name="fin", bufs=2) as finp,
            tc.tile_pool(name="pkq", bufs=2, space="PSUM") as pvp,
            tc.tile_pool(name="pv", bufs=1, space="PSUM") as pqp,
            tc.tile_pool(name="ps", bufs=CFG["psbufs"], space="PSUM") as psp,
            tc.tile_pool(name="pav", bufs=1, space="PSUM") as pavp,
        ):
            # t=0 preamble: memset a scratch line, then (a) a dummy exp on ACT
            # so the 1283ns activation-table load runs during the DMA wait,
            # (b) a chain of tiny PE matmuls so the PE p-state ramp (2.4GHz
            # only after 3us of continuous busy) burns off before real work.
            scratch = constp.tile([1, PB], BF16)
            nc.vector.memset(scratch[:, 0:H], 0)
            p_warm = pavp.tile([1, H], F32, tag="pav")
            for i in range(CFG["warmups"]):
                nc.tensor.matmul(
                    p_warm[:],
                    scratch[0:1, 0:1],
                    scratch[0:1, 0:H],
                    start=(i == 0),
                    stop=(i == CFG["warmups"] - 1),
                )

            # wqkv leads the sync queue in three slices (e0 / e1-3 / e4-7) so
            # the first slice's transfer beats the x tiles into the shared
            # DMA-engine FIFO and the later slices pipeline under them.
            wqkv_sb = constp.tile([PB, ET, WQP], BF16)
            for lo, hi in ((0, 1), (1, 4), (4, ET)):
                nc.sync.dma_start(
                    wqkv_sb[:, lo:hi, :],
                    wqkv[lo * PB : hi * PB, :].rearrange("(t p) m -> p t m", p=PB),
                )
            cs = constp.tile([PB, NCONST], BF16)
            bs = constp.tile([PB, NBIAS], F32)

            tri_ap = cs[:, C_TRI : C_TRI + PB]
            kb_ap = bs[0:H, CB_KQ : CB_KQ + 1]  # bk/8 at partitions 0:64
            qb_ap = bs[H:PB, CB_KQ : CB_KQ + 1]  # bq at partitions 64:128
            ones_ap = cs[:, C_ONES : C_ONES + 1]
            bvr_ap = cs[:, C_BVR : C_BVR + H]  # bv replicated per partition

            qT = qkvp.tile([H, S], BF16)  # q h-major
            kT = qkvp.tile([H, S], BF16)  # (k + bk)/8 h-major
            vsb = qkvp.tile([PB, KT, H + 1], BF16)  # v k-major + ones col

            def load_consts():
                # emitted after chunk 0's x tiles so the pool queue's first
                # deliveries are the tiles the first accumulation needs
                getattr(nc, CFG["cs_q"]).dma_start(cs[:], consts[:])
                getattr(nc, CFG["cs_q"]).dma_start(bs[:], biases[:])
                nc.vector.tensor_copy(
                    vsb[:, :, H : H + 1],
                    ones_ap[:, 0:1, None].to_broadcast((PB, KT, 1)),
                )
                # dummy exp: loads the ACT Exp table during the DMA wait
                nc.scalar.activation(scratch[0:1, H : H + 1], scratch[0:1, 0:1], AF.Exp)

            def proj_main(c):
                # [Wk|Wq] packed h-major pass (x moving) + v s-major pass
                # (x blocks stationary, Wv.T moving, output lands k-major =
                # the vsb layout directly). No PE transposes anywhere.
                qs = slice(c * QB, (c + 1) * QB)
                p_kq = pvp.tile([PB, QB], F32, tag="pkq")
                # v s-major; full-bank tile so its start's zero region (the
                # whole 2KB PSUM bank) can't clobber a co-resident tile
                p_v = pqp.tile([PB, QB], F32, tag="pv")
                for e in range(ET):
                    xt = xpool.tile([PB, QB], BF16, tag="xt")
                    if c == 0:
                        # sync is busy with the wqkv slices: x startup tiles
                        # ride scalar/gpsimd first, sync late
                        dma_eng = {
                            0: nc.scalar, 1: nc.scalar, 2: getattr(nc, CFG["dma2"]),
                            3: getattr(nc, CFG["dma2"]), 4: nc.scalar, 5: nc.sync,
                            6: getattr(nc, CFG["dma2"]), 7: nc.sync,
                        }[e]
                    else:
                        dma_eng = nc.sync if e % 2 == 0 else getattr(nc, CFG["dma2"])
                    dma_eng.dma_start(xt[:], xT[e * PB : (e + 1) * PB, qs])
                    nc.tensor.matmul(
                        p_kq[:],
                        wqkv_sb[:, e, 0:PB],
                        xt[:],
                        start=(e == 0),
                        stop=(e == ET - 1),
                    )
                    # v s-major: x block as stationary, Wv.T as moving. Four
                    # accumulation groups share one PSUM bank: only the very
                    # first matmul may set start (start zeroes the whole
                    # bank); groups 1-3's first writes land on still-pending
                    # zero bytes and store rather than accumulate.
                    for j in range(DIAG):
                        nc.tensor.matmul(
                            p_v[:, j * H : (j + 1) * H],
                            xt[:, j * PB : (j + 1) * PB],
                            wqkv_sb[:, e, 2 * H : 3 * H],
                            start=(e == 0 and j == 0),
                            stop=(e == ET - 1 and j == DIAG - 1),
                            skip_group_check=True,
                        )
                    yield
                # epilogue, all on DVE: k (pre-scaled by 1/8) and q gate the
                # next attention's scores; vsb blocks follow.
                nc.vector.tensor_scalar(
                    kT[:, qs], p_kq[0:H, :], 0.125, kb_ap, MUL, ADD
                )
                nc.vector.tensor_scalar(
                    qT[:, qs], p_kq[H:PB, :], qb_ap, None, ADD, mybir.AluOpType.bypass
                )
                yield
                for j in range(DIAG):
                    m = DIAG * c + j
                    nc.vector.tensor_tensor(
                        vsb[:, m, 0:H], p_v[:, j * H : (j + 1) * H], bvr_ap, ADD
                    )
                    if j % 2 == 1:
                        yield

            def attn(c):
                nkt = DIAG * c + DIAG
                p_av = pavp.tile([H + 1, QB], F32, tag="pav")

                def weights_pair(m):
                    # two full-width k-tiles share one 2-bank PSUM tile so a
                    # single exp covers both (one 185ns ACT init, not two)
                    p_s = psp.tile([PB, 2, QB], F32, tag="ps")
                    for h in range(2):
                        nc.tensor.matmul(
                            p_s[:, h, :],
                            kT[:, (m + h) * PB : (m + h + 1) * PB],
                            qT[:, c * QB : (c + 1) * QB],
                            start=True,
                            stop=True,
                        )
                    w = wtp2.tile([PB, 2, QB], BF16, tag="w2")
                    nc.scalar.activation(w[:], p_s[:], AF.Exp)
                    return w

                def weights_diag(m):
                    # diagonal k-tile: live columns only + triangle mask
                    lo = (m - DIAG * c) * PB
                    p_s = psp.tile([PB, 2, QB], F32, tag="ps")
                    nc.tensor.matmul(
                        p_s[:, 0, lo:QB],
                        kT[:, m * PB : (m + 1) * PB],
                        qT[:, c * QB + lo : (c + 1) * QB],
                        start=True,
                        stop=True,
                    )
                    w = wtp.tile([PB, QB], BF16, tag="w")
                    nc.scalar.activation(w[:, lo:QB], p_s[:, 0, lo:QB], AF.Exp)
                    nc.vector.tensor_tensor(
                        w[:, lo : lo + PB], w[:, lo : lo + PB], tri_ap, MUL
                    )
                    return w

                # units: (first_tile_m, n_tiles, generator) — pairs for the
                # full-width tiles, singles for the diagonal band
                units = [(m, 2, weights_pair) for m in range(0, DIAG * c, 2)]
                diag_units = [(m, 1, weights_diag) for m in range(DIAG * c, nkt)]
                if c == QC - 1 and CFG["diag_first"]:
                    # final chunk: diagonals first so the drain of the last
                    # (unpipelined) m-steps has no exp->mask->AV chain
                    units = diag_units + units
                else:
                    units = units + diag_units
                L = CFG["lookahead"]
                ws = {u[0]: u[2](u[0]) for u in units[: min(L, len(units))]}
                yield
                n_av = 0
                for idx, (m, n, _gen) in enumerate(units):
                    if idx + L < len(units):
                        un = units[idx + L]
                        ws[un[0]] = un[2](un[0])
                    w = ws.pop(m)
                    for h in range(n):
                        lo = 0 if n == 2 else (m - DIAG * c) * PB
                        wap = w[:, h, lo:QB] if n == 2 else w[:, lo:QB]
                        nc.tensor.matmul(
                            p_av[:, lo:QB],
                            vsb[:, m + h, :],
                            wap,
                            start=(n_av == 0),
                            stop=(n_av == nkt - 1),
                        )
                        n_av += 1
                    yield
                # unnormalized output + denominator row; division happens on
                # the host as part of unsharding. The last chunk goes out in
                # column halves, copies on two engines, both DMAs early.
                osb = finp.tile([H + 1, QB], F32, tag="osb")
                if c == QC - 1:
                    hw_ = QB // 2
                    nc.vector.tensor_copy(osb[:, 0:hw_], p_av[:, 0:hw_])
                    nc.gpsimd.tensor_copy(osb[:, hw_:QB], p_av[:, hw_:QB])
                    nc.sync.dma_start(out[:, c * QB : c * QB + hw_], osb[:, 0:hw_])
                    nc.scalar.dma_start(
                        out[:, c * QB + hw_ : (c + 1) * QB], osb[:, hw_:QB]
                    )
                    yield
                else:
                    nc.vector.tensor_copy(osb[:], p_av[:])
                    yield
                    nc.sync.dma_start(out[:, c * QB : (c + 1) * QB], osb[:])
                    yield

            # interleaved emission: proj_main(c) alternates with attn(c-1) so
            # the in-order engine queues see attention work during DMA waits;
            # each projection epilogue is emitted after that attention so no
            # exp/mask queues behind an epilogue copy still waiting on DMA.
            g0 = proj_main(0)
            for _ in range(2):
                next(g0)  # chunk 0's first x tiles lead the DMA queues
            load_consts()
            _interleave(g0)
            for c in range(1, QC):
                _interleave(proj_main(c), attn(c - 1))
            _interleave(attn(QC - 1))

    nc.compile()
    return nc


def _host_inputs(x, Wq, bq, Wk, bk, Wv, bv):
    bf16 = ml_dtypes.bfloat16
    x = np.asarray(x, np.float32)
    Wq, bq = np.asarray(Wq, np.float32), np.asarray(bq, np.float32)
    Wk, bk = np.asarray(Wk, np.float32), np.asarray(bk, np.float32)
    Wv, bv = np.asarray(Wv, np.float32), np.asarray(bv, np.float32)

    wqkv = np.zeros((E, WQP), np.float32)  # padded to 512B rows for full DMA rate
    wqkv[:, 0 : 3 * H] = np.concatenate([Wk.T, Wq.T, Wv.T], axis=1)
    wqkv = wqkv.astype(bf16)

    cs = np.zeros((PB, NCONST), np.float32)
    jj = np.arange(PB, dtype=np.int64)[None, :]
    pp = np.arange(PB, dtype=np.int64)[:, None]
    cs[:, C_TRI : C_TRI + PB] = (jj >= pp).astype(np.float32)
    cs[:, C_ONES] = 1.0
    cs[:, C_BVR : C_BVR + H] = bv[None, :]
    cs = cs.astype(bf16)

    bsc = np.zeros((PB, NBIAS), np.float32)
    bsc[:H, CB_KQ] = bk * 0.125
    bsc[H:PB, CB_KQ] = bq

    shared = {"wqkv": wqkv, "consts": cs, "biases": bsc}
    in_maps = []
    for b in range(B):
        m = dict(shared)
        m["xT"] = np.ascontiguousarray(x[b].T).astype(bf16)
        in_maps.append(m)
    return in_maps


def get_nc():
    if "nc" not in _CACHE:
        _CACHE["nc"] = _build_nc()
    return _CACHE["nc"]


def kernel(x, Wq, bq, Wk, bk, Wv, bv):
    nc = get_nc()
    in_maps = _host_inputs(x, Wq, bq, Wk, bk, Wv, bv)
    results = bass2jax.run_bass_via_pjrt(nc, in_maps, n_cores=NCORES)
    out = np.empty((B, S, H), np.float32)
    for b in range(B):
        o = results[b]["out"]
        out[b] = (o[:H] / o[H : H + 1]).T
    return out


# revision 57
# speedup vs baseline: 1.3360x; 1.0177x over previous
"""Single-head causal attention on 8 Trainium2 NeuronCores.

Problem: x:[8,2048,1024], Wq/Wk/Wv:[64,1024], bq/bk/bv:[64]
  q,k,v = x@W*.T + b*;  out = softmax(causal(q@k.T)/sqrt(64)) @ v

Sharding: batch dim (8) across the 8 cores - fully data-parallel, no
collectives. Each core computes one batch's attention head.

v2 design (all matmuls bf16, 1 cycle/row on the PE at any free size):
  - x and the packed projection weights are converted to bf16 on the host:
    halves HBM traffic and frees the fp32r >=256-free-dim constraint.
  - k/v projection stays h-major ([Wk|Wv] packed stationary, x moving:
    full 128-wide output). q projection runs s-major: x tiles double as
    stationary [128e,128s] blocks, Wq.T [128e,64] is the moving operand -
    64 rows per (e,s) pair instead of 512 per (e,chunk), 2x fewer PE rows
    for q. q then transposes back to h-major on the PE (bf16 identity).
  - causal dead-column skipping: for diagonal k-tiles only columns
    [128i, 512) are live; scores/exp/AV all operate on the live slice
    (saves ~2.5us PE + ~2.5us ACT). Only the [128,128] triangle block
    needs the 0/1 mask multiply (DVE, bf16).
  - ACT runs exp almost exclusively (the bottleneck engine in v1 at 82%);
    q/k epilogue copies live on DVE, v on ACT, outputs DMA straight from
    PSUM (no SBUF staging).
  - softmax without max-subtraction (scores/8 ~ N(0,1); exp safe in f32),
    denominator = ones-row appended to V, division on the host.
  - emission interleaves proj(c+1) with attn(c); the last chunk processes
    its (cheap, live-sliced) diagonal tiles first so the drain is fed by
    exps computed well in advance.
"""

import numpy as np
import ml_dtypes

import concourse.bacc as bacc
import concourse.mybir as mybir
import concourse.tile as tile
from concourse import bass2jax

B, S, E, H = 8, 2048, 1024, 64
NCORES = 8
PB = 128  # partition block / k-tile size
QB = 512  # q-chunk (matmul moving free dim)
ET = E // PB  # e-tiles per contraction
QC = S // QB  # q-chunks
KT = S // PB  # k-tiles
DIAG = QB // PB  # diagonal k-tiles per q-chunk

# q-chunk decomposition (start, width)
CHUNKS = [(0, 512), (512, 512), (1024, 512), (1536, 512)]

# packed bf16 constants layout: columns of the [128, NCONST] "consts" input
C_TRI = 0  # [*, 0:128]    causal triangle: (p, j) = 1 iff j >= p
C_ONES = 128  # [*, 128:129]  ones
C_BVR = 129  # [*, 129:193]  v bias replicated on every partition row
NCONST = 193
WQP = 2 * PB  # wqkv padded row length (bf16 512B descriptors, full DMA rate)
# f32 bias constants: columns of the [128, NBIAS] "biases" input
CB_KQ = 0  # k bias / 8 (rows 0:64), q bias (rows 64:128)
NBIAS = 1

F32 = mybir.dt.float32
BF16 = mybir.dt.bfloat16
AF = mybir.ActivationFunctionType
MUL = mybir.AluOpType.mult
ADD = mybir.AluOpType.add

_CACHE: dict = {}

# schedule/buffering knobs (sweepable)
CFG = {
    "lookahead": 5,
    "xbufs": 8,
    "wtbufs": 6,
    "wt2bufs": 5,
    "psbufs": 2,  # [128, 2, 512] double-bank score tiles
    "dma2": "gpsimd",  # second x-stream queue
    "cs_q": "gpsimd",
    "diag_first": False,
    "warmups": 38,  # PE p-state warmup matmuls (64-free) during startup DMA
}


def _interleave(*gens):
    """Drive generators round-robin; the first (proj) gets two steps per turn."""
    alive = list(gens)
    steps = {id(g): (2 if i == 0 and len(gens) > 1 else 1) for i, g in enumerate(gens)}
    while alive:
        for g in list(alive):
            for _ in range(steps[id(g)]):
                try:
                    next(g)
                except StopIteration:
                    alive.remove(g)
                    break


def _build_nc():
    nc = bacc.Bacc("TRN2", target_bir_lowering=False, debug=False)
    xT = nc.dram_tensor("xT", [E, S], BF16, kind="ExternalInput").ap()
    # cols 0:64 Wk.T, 64:128 Wv.T, 128:192 Wq.T, 192:256 zero padding
    wqkv = nc.dram_tensor("wqkv", [E, WQP], BF16, kind="ExternalInput").ap()
    consts = nc.dram_tensor("consts", [PB, NCONST], BF16, kind="ExternalInput").ap()
    biases = nc.dram_tensor("biases", [PB, NBIAS], F32, kind="ExternalInput").ap()
    out = nc.dram_tensor("out", [H + 1, S], F32, kind="ExternalOutput").ap()

    with tile.TileContext(nc) as tc:
        with (
            tc.tile_pool(name="const", bufs=1) as constp,
            tc.tile_pool(name="xs", bufs=CFG["xbufs"]) as xpool,
            tc.tile_pool(name="qkv", bufs=1) as qkvp,
            tc.tile_pool(name="wt", bufs=CFG["wtbufs"]) as wtp,
            tc.tile_pool(name="wt2", bufs=CFG["wt2bufs"]) as wtp2,
            tc.tile_pool(name="fin", bufs=2) as finp,
            tc.tile_pool(name="pkq", bufs=2, space="PSUM") as pvp,
            tc.tile_pool(name="pv", bufs=1, space="PSUM") as pqp,
            tc.tile_pool(name="ps", bufs=CFG["psbufs"], space="PSUM") as psp,
            tc.tile_pool(name="pav", bufs=1, space="PSUM") as pavp,
        ):
            # t=0 preamble: memset a scratch line, then (a) a dummy exp on ACT
            # so the 1283ns activation-table load runs during the DMA wait,
            # (b) a chain of tiny PE matmuls so the PE p-state ramp (2.4GHz
            # only after 3us of continuous busy) burns off before real work.
            scratch = constp.tile([1, PB], BF16)
            nc.vector.memset(scratch[:, 0:H], 0)
            p_warm = pavp.tile([1, H], F32, tag="pav")
            for i in range(CFG["warmups"]):
                nc.tensor.matmul(
                    p_warm[:],
                    scratch[0:1, 0:1],
                    scratch[0:1, 0:H],
                    start=(i == 0),
                    stop=(i == CFG["warmups"] - 1),
                )

            # wqkv: the e0 slice leads the sync queue (first matmul's input);
            # the rest rides scalar, whose queue is otherwise idle at startup.
            # Transfers on different queues run in parallel; only same-queue
            # transfers serialize.
            wqkv_sb = constp.tile([PB, ET, WQP], BF16)
            nc.sync.dma_start(
                wqkv_sb[:, 0:1, :],
                wqkv[0:PB, :].rearrange("(t p) m -> p t m", p=PB),
            )
            nc.scalar.dma_start(
                wqkv_sb[:, 1:ET, :],
                wqkv[PB:, :].rearrange("(t p) m -> p t m", p=PB),
            )
            cs = constp.tile([PB, NCONST], BF16)
            bs = constp.tile([PB, NBIAS], F32)

            tri_ap = cs[:, C_TRI : C_TRI + PB]
            kb_ap = bs[0:H, CB_KQ : CB_KQ + 1]  # bk/8 at partitions 0:64
            qb_ap = bs[H:PB, CB_KQ : CB_KQ + 1]  # bq at partitions 64:128
            ones_ap = cs[:, C_ONES : C_ONES + 1]
            bvr_ap = cs[:, C_BVR : C_BVR + H]  # bv replicated per partition

            qT = qkvp.tile([H, S], BF16)  # q h-major
            kT = qkvp.tile([H, S], BF16)  # (k + bk)/8 h-major
            vsb = qkvp.tile([PB, KT, H + 1], BF16)  # v k-major + ones col

            def load_consts():
                # emitted after chunk 0's x tiles so the pool queue's first
                # deliveries are the tiles the first accumulation needs
                getattr(nc, CFG["cs_q"]).dma_start(cs[:], consts[:])
                getattr(nc, CFG["cs_q"]).dma_start(bs[:], biases[:])
                nc.vector.tensor_copy(
                    vsb[:, :, H : H + 1],
                    ones_ap[:, 0:1, None].to_broadcast((PB, KT, 1)),
                )
                # dummy exp: loads the ACT Exp table during the DMA wait
                nc.scalar.activation(scratch[0:1, H : H + 1], scratch[0:1, 0:1], AF.Exp)

            def proj_main(ci):
                # [Wk|Wq] packed h-major pass (x moving) + v s-major pass
                # (x blocks stationary, Wv.T moving, output lands k-major =
                # the vsb layout directly). No PE transposes anywhere.
                c0, w = CHUNKS[ci]
                nj = w // PB  # v s-major blocks
                qs = slice(c0, c0 + w)
                p_kq = pvp.tile([PB, QB], F32, tag="pkq")
                # v s-major; full-bank tile so its start's zero region (the
                # whole 2KB PSUM bank) can't clobber a co-resident tile
                p_v = pqp.tile([PB, QB], F32, tag="pv")
                # x arrives as two quad tiles (4 e-slices per dma_start), one
                # per queue: big transfers keep both queues transfer-bound
                # (gpsimd's ~1us SWDGE dispatch otherwise throttles its
                # stream) and the two transfers run in parallel.
                xq = []
                for g in range(2):
                    t = xpool.tile([PB, ET // 2, QB], BF16, tag="xt")
                    eng = nc.sync if g == 0 else getattr(nc, CFG["dma2"])
                    eng.dma_start(
                        t[:, :, 0:w],
                        xT[g * 4 * PB : (g + 1) * 4 * PB, qs].rearrange(
                            "(t p) m -> p t m", p=PB
                        ),
                    )
                    xq.append(t)
                for e in range(ET):
                    xt = xq[e // 4][:, e % 4, :]
                    nc.tensor.matmul(
                        p_kq[:, 0:w],
                        wqkv_sb[:, e, 0:PB],
                        xt[0:PB, 0:w],
                        start=(e == 0),
                        stop=(e == ET - 1),
                    )
                    # v s-major: x block as stationary, Wv.T as moving. The
                    # accumulation groups share one PSUM bank: only the very
                    # first matmul may set start (start zeroes the whole
                    # bank); later groups' first writes land on still-pending
                    # zero bytes and store rather than accumulate.
                    for j in range(nj):
                        nc.tensor.matmul(
                            p_v[:, j * H : (j + 1) * H],
                            xt[0:PB, j * PB : (j + 1) * PB],
                            wqkv_sb[:, e, 2 * H : 3 * H],
                            start=(e == 0 and j == 0),
                            stop=(e == ET - 1 and j == nj - 1),
                            skip_group_check=True,
                        )
                    yield
                # epilogue, all on DVE: q first — the next attention's pair
                # scores read old kT chunks + the fresh qT, so q alone
                # unblocks them; k (pre-scaled by 1/8) only gates the
                # diagonal tiles.
                nc.vector.tensor_scalar(
                    qT[:, qs], p_kq[H:PB, 0:w], qb_ap, None, ADD, mybir.AluOpType.bypass
                )
                nc.vector.tensor_scalar(
                    kT[:, qs], p_kq[0:H, 0:w], 0.125, kb_ap, MUL, ADD
                )
                yield
                for j in range(nj):
                    m = c0 // PB + j
                    nc.vector.tensor_tensor(
                        vsb[:, m, 0:H], p_v[:, j * H : (j + 1) * H], bvr_ap, ADD
                    )
                    if j % 2 == 1:
                        yield

            def attn(ci):
                c0, w = CHUNKS[ci]
                m0 = c0 // PB  # first diagonal k-tile
                nkt = (c0 + w) // PB
                p_av = pavp.tile([H + 1, QB], F32, tag="pav")

                def weights_pair(m):
                    # two full-width k-tiles share one 2-bank PSUM tile so a
                    # single exp covers both (one 185ns ACT init, not two)
                    p_s = psp.tile([PB, 2, QB], F32, tag="ps")
                    for h in range(2):
                        nc.tensor.matmul(
                            p_s[:, h, 0:w],
                            kT[:, (m + h) * PB : (m + h + 1) * PB],
                            qT[:, c0 : c0 + w],
                            start=True,
                            stop=True,
                        )
                    wt = wtp2.tile([PB, 2, QB], BF16, tag="w2")
                    nc.scalar.activation(wt[:, :, 0:w], p_s[:, :, 0:w], AF.Exp)
                    return wt

                def weights_diag(m):
                    # diagonal k-tile: live columns only + triangle mask
                    lo = (m - m0) * PB
                    p_s = psp.tile([PB, 2, QB], F32, tag="ps")
                    nc.tensor.matmul(
                        p_s[:, 0, lo:w],
                        kT[:, m * PB : (m + 1) * PB],
                        qT[:, c0 + lo : c0 + w],
                        start=True,
                        stop=True,
                    )
                    wt = wtp.tile([PB, QB], BF16, tag="w")
                    nc.scalar.activation(wt[:, lo:w], p_s[:, 0, lo:w], AF.Exp)
                    nc.vector.tensor_tensor(
                        wt[:, lo : lo + PB], wt[:, lo : lo + PB], tri_ap, MUL
                    )
                    return wt

                # units: (first_tile_m, n_tiles, generator) — pairs for the
                # full-width tiles, singles for the diagonal band
                units = [(m, 2, weights_pair) for m in range(0, m0, 2)]
                diag_units = [(m, 1, weights_diag) for m in range(m0, nkt)]
                if ci == len(CHUNKS) - 1 and CFG["diag_first"]:
                    # final chunk: diagonals first so the drain of the last
                    # (unpipelined) m-steps has no exp->mask->AV chain
                    units = diag_units + units
                else:
                    units = units + diag_units
                L = CFG["lookahead"]
                ws = {u[0]: u[2](u[0]) for u in units[: min(L, len(units))]}
                yield
                n_av = 0
                for idx, (m, n, _gen) in enumerate(units):
                    if idx + L < len(units):
                        un = units[idx + L]
                        ws[un[0]] = un[2](un[0])
                    wt = ws.pop(m)
                    for h in range(n):
                        lo = 0 if n == 2 else (m - m0) * PB
                        wap = wt[:, h, lo:w] if n == 2 else wt[:, lo:w]
                        nc.tensor.matmul(
                            p_av[:, lo:w],
                            vsb[:, m + h, :],
                            wap,
                            start=(n_av == 0),
                            stop=(n_av == nkt - 1),
                        )
                        n_av += 1
                    yield
                # unnormalized output + denominator row; division happens on
                # the host as part of unsharding. The last chunk drains in
                # 128-column pieces: with the diagonal units ordered last,
                # piece j's columns are complete after diag-j's AV, so the
                # copies/DMAs pipeline under the remaining diagonal AVs
                # (gpsimd can't read PSUM: copies alternate DVE/ACT).
                osb = finp.tile([H + 1, QB], F32, tag="osb")
                if ci == len(CHUNKS) - 1:
                    hw_ = w // 2
                    nc.vector.tensor_copy(osb[:, 0:hw_], p_av[:, 0:hw_])
                    # gpsimd can't read PSUM; ACT is idle after the last exp
                    nc.scalar.activation(osb[:, hw_:w], p_av[:, hw_:w], AF.Copy)
                    nc.sync.dma_start(out[:, c0 : c0 + hw_], osb[:, 0:hw_])
                    nc.sync.dma_start(out[:, c0 + hw_ : c0 + w], osb[:, hw_:w])
                    yield
                else:
                    nc.vector.tensor_copy(osb[:, 0:w], p_av[:, 0:w])
                    yield
                    nc.sync.dma_start(out[:, c0 : c0 + w], osb[:, 0:w])
                    yield

            # interleaved emission: proj_main(c) alternates with attn(c-1) so
            # the in-order engine queues see attention work during DMA waits;
            # each projection epilogue is emitted after that attention so no
            # exp/mask queues behind an epilogue copy still waiting on DMA.
            g0 = proj_main(0)
            for _ in range(2):
                next(g0)  # chunk 0's first x tiles lead the DMA queues
            load_consts()
            _interleave(g0)
            for ci in range(1, len(CHUNKS)):
                _interleave(proj_main(ci), attn(ci - 1))
            _interleave(attn(len(CHUNKS) - 1))

    nc.compile()
    return nc


def _host_inputs(x, Wq, bq, Wk, bk, Wv, bv):
    bf16 = ml_dtypes.bfloat16
    x = np.asarray(x, np.float32)
    Wq, bq = np.asarray(Wq, np.float32), np.asarray(bq, np.float32)
    Wk, bk = np.asarray(Wk, np.float32), np.asarray(bk, np.float32)
    Wv, bv = np.asarray(Wv, np.float32), np.asarray(bv, np.float32)

    wqkv = np.zeros((E, WQP), np.float32)  # padded to 512B rows for full DMA rate
    wqkv[:, 0 : 3 * H] = np.concatenate([Wk.T, Wq.T, Wv.T], axis=1)
    wqkv = wqkv.astype(bf16)

    cs = np.zeros((PB, NCONST), np.float32)
    jj = np.arange(PB, dtype=np.int64)[None, :]
    pp = np.arange(PB, dtype=np.int64)[:, None]
    cs[:, C_TRI : C_TRI + PB] = (jj >= pp).astype(np.float32)
    cs[:, C_ONES] = 1.0
    cs[:, C_BVR : C_BVR + H] = bv[None, :]
    cs = cs.astype(bf16)

    bsc = np.zeros((PB, NBIAS), np.float32)
    bsc[:H, CB_KQ] = bk * 0.125
    bsc[H:PB, CB_KQ] = bq

    shared = {"wqkv": wqkv, "consts": cs, "biases": bsc}
    in_maps = []
    for b in range(B):
        m = dict(shared)
        m["xT"] = np.ascontiguousarray(x[b].T).astype(bf16)
        in_maps.append(m)
    return in_maps


def get_nc():
    if "nc" not in _CACHE:
        _CACHE["nc"] = _build_nc()
    return _CACHE["nc"]


def kernel(x, Wq, bq, Wk, bk, Wv, bv):
    nc = get_nc()
    in_maps = _host_inputs(x, Wq, bq, Wk, bk, Wv, bv)
    results = bass2jax.run_bass_via_pjrt(nc, in_maps, n_cores=NCORES)
    out = np.empty((B, S, H), np.float32)
    for b in range(B):
        o = results[b]["out"]
        out[b] = (o[:H] / o[H : H + 1]).T
    return out
